# revision 38
# baseline (speedup 1.0000x reference)
"""Multi-Head Latent Attention (MLA) on 8 Trainium2 NeuronCores.

Sharding: core = b*4 + hg, b in {0,1} batch, hg in 0..3 head-groups of 4
heads (512 of the 2048 d_out dims). The latent projections (c_kv) are
computed per-core; the low-rank Q path is absorbed ON HOST:
    W_effQ^T = W_DQ^T @ W_UQ_shard^T   ([d_in, 512])
(a weights-only transform), so the device does q_shard = x_b @ W_effQ as
one 2048-contraction matmul and never sees W_DQ/W_UQ.

Everything on device lives in transposed "feature-on-partition" layout:
  XT = x[b]^T [d_in, N], QT = q^T, CKT = c_kv^T, KT = k^T. Attention
computes S^T tiles [ktok, qtok] directly (matmul lhsT=KT-slice,
rhs=QT-slice), so softmax probabilities come out of exp already in the
layout the ctx matmul needs (contraction over ktok on partitions) — no
PE transposes. Causality: affine_select zeroes P^T[kj, q] for kj > q
after exp (no max-subtraction needed: scores are O(1) by construction).

The softmax denominator is NOT a per-tile PE matmul: exp tiles (fp16)
are accumulated on the DVE (fp16 all-2-byte => 4x mode), then ONE
all-ones [128,128] matmul per (group, head) broadcasts the partition
sums to every partition; reciprocal+multiply normalize ctx^T straight
into the per-(g,h) normalized-ctx tile the output matmul reads.

Scheduling: the attention inner loop is paced by the scalar-engine exp
(~0.7us per [128,512] tile) while its own PE work (S+ctx) is only
~0.43us. A filler queue of projection-chunk and output-chunk closures
is drained between attention steps on a ns budget, so the PE stays fed
during the scalar-bound attention stretches instead of idling.

Output per core: partial out^T [d_in, N] (contraction over this core's
512 ctx dims); host sums the 4 head-group partials per batch and adds
the bias.
"""

import math
from collections import deque
from contextlib import ExitStack

import numpy as np

import concourse.bass as bass
import concourse.bass_isa as bass_isa
import concourse.mybir as mybir
import concourse.tile as tile
from concourse.bass_utils import run_bass_kernel_spmd
from concourse.vector_clock import ScopedClock, VectorClock

FP32 = mybir.dt.float32
BF16 = mybir.dt.bfloat16
FP16 = mybir.dt.float16
FP8 = mybir.dt.float8e4
P = 128
CH = 512
CAST_SPLIT = True
# The absorbed Q weights are scaled by QSCALE before the fp8e4 cast so they
# sit in e4m3's normal range (raw std ~0.018 straddles the 2^-6 subnormal
# cutoff); the 1/QSCALE is folded into the softmax exp scale.
QSCALE = 64.0


class SplitDrainTileContext(tile.TileContext):
    """TileContext whose tail drain splits sem waits across multiple NOPs.

    The walrus build in this container rejects instructions carrying >2
    sync waits ("Too many sync wait commands"); stock TileContext puts a
    wait for every outstanding proc on the single kernel-tail drain.
    """

    def _drain_and_barrier(self, tick_clock, wait_clock):
        g = tick_clock.global_clock
        n = len(g)
        for i in range(n):
            t = g[i]
            if t <= 0:
                continue
            vc = VectorClock([0] * n)
            vc.require_at_least(i, t)
            nop = self.nc.sync.nop(hint="split_drain_wait", nofuse=True)
            wait_clock.add_sem_waits(nop.ins, ScopedClock({None: vc}))
        self.nc.sync.drain()
        self.nc.all_engine_barrier()
        assert self.sems is not None
        popped = self.nc._tile_sem_poison_stack.pop()
        assert popped is self._sem_poison
        self.nc.clear_and_free_semaphores(list(self.sems.allocated().values()))
        self.nc.all_engine_barrier()


def split_multi_waits(nc, max_waits=1):
    """Hoist extra sync waits onto same-engine NOPs.

    The walrus build here rejects instructions with more than ~2 sync wait
    commands; Tile freely attaches one wait per outstanding proc. An engine
    executes its stream in order, so a NOP carrying a wait immediately
    before the instruction is semantically identical.
    """
    for fn in nc.m.functions:
        for bb in fn.blocks:
            new_insts = []
            changed = False
            for inst in bb.instructions:
                si = inst.sync_info
                waits = list(si.on_wait) if si is not None else []
                if len(waits) > max_waits:
                    extra, keep = waits[:-max_waits], waits[-max_waits:]
                    for k, w in enumerate(extra):
                        nop = mybir.InstNoOp(
                            name=f"{inst.name}.w{k}",
                            sync_info=mybir.SyncInfo(on_wait=[w], on_update=[]),
                            bass_nofuse=True,
                            engine=inst.engine,
                        )
                        new_insts.append(nop)
                    inst.sync_info = mybir.SyncInfo(
                        on_wait=keep, on_update=list(si.on_update)
                    )
                    changed = True
                new_insts.append(inst)
            if changed:
                bb.instructions = new_insts


def build_nc(N=2048, D=2048, KV=512, HC=4, DH=128, split=True):
    """Build the per-core Bass program (identical on all 8 cores)."""
    HD = HC * DH  # this core's slice of d_out
    n_ct = D // P  # d_in partition tiles
    n_klt = KV // P  # kv-latent tiles
    n_ht = HD // P  # head tiles (DH == P so one tile per head)
    n_ch = N // CH  # token chunks
    kpc = CH // P  # ktiles per chunk (4)
    scale = 1.0 / math.sqrt(DH)
    assert DH == P and n_ct % 4 == 0

    nc = bass.Bass("TRN2", target_bir_lowering=False, debug=False)
    xt = nc.declare_dram_parameter("xt", [D, N], BF16, isOutput=False)
    # Q path in fp8e4 DoubleRow (2x PE throughput, 256-deep contraction per
    # matmul). q errors only reach the output through softmax probabilities,
    # so fp8's ~3.7% q-noise decays to ~1% on the final output — well under
    # the 2e-2 gate. x/weffq ship as separate fp8 copies from the host.
    xf8 = nc.declare_dram_parameter("xf8", [D, N], FP8, isOutput=False)
    weffqf8 = nc.declare_dram_parameter("weffqf8", [D, HD], FP8, isOutput=False)
    wdkvt = nc.declare_dram_parameter("wdkvt", [D, KV], BF16, isOutput=False)
    # K path also fp8 (k errors, like q errors, only reach the output
    # through softmax probabilities); W_UK ships x QSCALE like weffq
    wukt = nc.declare_dram_parameter("wukt", [KV, HD], FP8, isOutput=False)
    wuvt = nc.declare_dram_parameter("wuvt", [KV, HD], BF16, isOutput=False)
    wot = nc.declare_dram_parameter("wot", [HD, D], BF16, isOutput=False)
    outt = nc.declare_dram_parameter("outt", [D, N], BF16, isOutput=True)

    with SplitDrainTileContext(nc) as tc, ExitStack() as top:
        mm = nc.tensor.matmul

        # ~4us of dummy matmuls at kernel start: trips the HAM activity
        # window so the real matmuls start at 2.4GHz instead of 1.2.
        const = top.enter_context(tc.tile_pool(name="const", bufs=1))
        # all-ones square: one matmul broadcasts the partition-sum of the
        # softmax-denominator accumulator to every partition
        # (gpsimd.partition_all_reduce would free the PE here, but this
        # walrus build rejects InstPartitionAllReduce: "ISA wrong length")
        ones_sq = const.tile([P, P], FP16, tag="ones_sq", name="ones_sq")
        nc.vector.memset(ones_sq, 1.0)
        # Dummy matmuls trip the HAM activity window so real matmuls run at
        # full speed. The startup is DMA-bound (x0+weights stream at
        # ~350GB/s until ~25us), and any PE-idle hole >~2us mid-startup
        # makes HAM drop back to half clock — so beyond this upfront block,
        # more dummies are PADDED between the DMA-gated g4-blocks of the
        # first QT pass (see x_closures pad hook below).
        N_WARM = 10
        warm = const.tile([P, CH], BF16, tag="warm", name="warm")
        nc.vector.memset(warm, 0.0)
        with tc.tile_pool(name="psWarm", bufs=1, space="PSUM") as psWarm:
            wps = psWarm.tile([P, CH], FP32, tag="wps", name="wps")
            for i in range(N_WARM):
                mm(wps, lhsT=warm[:, :P], rhs=warm,
                   start=(i == 0), stop=(i == N_WARM - 1))

        # whole-kernel residents (bf16/fp16 keeps this under SBUF budget)
        kvp = top.enter_context(tc.tile_pool(name="kv", bufs=1))
        kt_sb = [
            kvp.tile([P, N], BF16, tag=f"kt{h}", name=f"kt{h}") for h in range(n_ht)
        ]
        v_sb = [
            kvp.tile([P, HD], FP16, tag=f"v{t}", name=f"v{t}")
            for t in range(N // P)
        ]
        qt_res = [
            kvp.tile([P, N], BF16, tag=f"qt{h}", name=f"qt{h}") for h in range(n_ht)
        ]

        # weights as single wide tiles: batched DMAs (each dma_start costs
        # ~600ns of Sync-engine issue time, so fewer+bigger is better)
        wp = top.enter_context(tc.tile_pool(name="wp", bufs=1))
        weffqf8_sb = wp.tile([P, n_ct, HD], FP8, tag="weffqf8", name="weffqf8")
        wdkvt_sb = wp.tile([P, n_ct, KV], BF16, tag="wdkvt", name="wdkvt")
        wukt_sb = wp.tile([P, n_klt, HD], FP8, tag="wukt", name="wukt")
        wuvt_sb = wp.tile([P, n_klt, HD], BF16, tag="wuvt", name="wuvt")
        wot_sb = wp.tile([P, n_ht, D], BF16, tag="wot", name="wot")

        def r128(ap):
            return ap.rearrange("(a p) c -> p a c", p=P)

        with (
            tc.tile_pool(name="xtp", bufs=2) as xtp,
            tc.tile_pool(name="xf8p", bufs=2) as xf8p,
            tc.tile_pool(name="cktp", bufs=2) as cktp,
            tc.tile_pool(name="ckt8p", bufs=2) as ckt8p,
            tc.tile_pool(name="ptp", bufs=6) as ptp,
            tc.tile_pool(name="accp", bufs=2) as accp,
            tc.tile_pool(name="bcp", bufs=2) as bcp,
            tc.tile_pool(name="otp", bufs=4) as otp,
            tc.tile_pool(name="oop", bufs=2) as oop,
            tc.tile_pool(name="psX", bufs=2, space="PSUM") as psX,
            tc.tile_pool(name="psS", bufs=3, space="PSUM") as psS,
            tc.tile_pool(name="psA", bufs=2, space="PSUM") as psA,
            tc.tile_pool(name="psN", bufs=1, space="PSUM") as psN,
        ):
            def dma_xt(ch):
                tok = slice(ch * CH, (ch + 1) * CH)
                xts = xtp.tile([P, n_ct, CH], BF16, tag="xts", name=f"xts{ch}")
                xf8s = xf8p.tile([P, n_ct, CH], FP8, tag="xf8s",
                                 name=f"xf8s{ch}")
                nc.sync.dma_start(out=xf8s, in_=r128(xf8[:, tok]))
                for g4 in range(0, n_ct, 4):
                    nc.sync.dma_start(
                        out=xts[:, g4 : g4 + 4, :],
                        in_=r128(xt[g4 * P : (g4 + 4) * P, tok]),
                    )
                return xts, xf8s

            # startup DMAs, dual HWDGE streams (sync | scalar), QT-critical
            # data first — only 2MB of fp8 before the first QT pass can run:
            #   sync:   xf8(chunk0) -> xt(chunk0) -> wukt -> chunk1 -> wot
            #   scalar: weffqf8     -> wdkvt      -> wuvt
            xts01 = {}
            xts0 = xtp.tile([P, n_ct, CH], BF16, tag="xts", name="xts0")
            xf80 = xf8p.tile([P, n_ct, CH], FP8, tag="xf8s", name="xf8s0")
            nc.sync.dma_start(out=xf80, in_=r128(xf8[:, 0:CH]))
            for g4 in range(0, n_ct, 4):
                nc.scalar.dma_start(
                    out=weffqf8_sb[:, g4 : g4 + 4, :],
                    in_=r128(weffqf8[g4 * P : (g4 + 4) * P, :]),
                )
            for g4 in range(0, n_ct, 4):
                nc.sync.dma_start(
                    out=xts0[:, g4 : g4 + 4, :],
                    in_=r128(xt[g4 * P : (g4 + 4) * P, 0:CH]),
                )
                nc.scalar.dma_start(
                    out=wdkvt_sb[:, g4 : g4 + 4, :],
                    in_=r128(wdkvt[g4 * P : (g4 + 4) * P, :]),
                )
            nc.sync.dma_start(out=wukt_sb, in_=r128(wukt[:, :]))
            nc.scalar.dma_start(out=wuvt_sb, in_=r128(wuvt[:, :]))
            xts01[0] = (xts0, xf80)
            xts01[1] = dma_xt(1)
            nc.sync.dma_start(out=wot_sb, in_=r128(wot[:, :]))

            MM_NS = 0.43  # ns per moving column, one 128-contraction matmul

            def cast(alt, out, in_):
                # split PSUM->SBUF casts between the two copy-capable
                # engines; Copy shares the exp activation table, so no
                # table reloads are triggered on the scalar engine
                if alt and CAST_SPLIT:
                    nc.scalar.copy(out=out, in_=in_)
                else:
                    nc.vector.tensor_copy(out=out, in_=in_)

            # ---- projection chunk as filler closures --------------------
            def x_closures(ch, pre=None):
                """Closures computing QT/CKT/KT/V for token chunk ch."""
                tok = slice(ch * CH, (ch + 1) * CH)
                st = {}
                out = []

                def open_chunk():
                    st["xts"], st["xf8"] = pre if pre is not None else dma_xt(ch)

                out.append((0, open_chunk))

                # QT then CKT: SINGLE-accumulator passes over the 16 d_in
                # tiles -- a new pass's psX alloc then sits a full pass
                # (~3.5us) behind the previous pass's final cast, so the
                # pool rotation never stalls the PE at pass boundaries
                def mk_proj(key, qi, ct, wt, res_write):
                    def f():
                        if ct == 0:
                            st[key] = psX.tile(
                                [P, CH], FP32, tag="psX", name=key
                            )
                        mm(
                            st[key],
                            lhsT=wt[:, ct, qi * P : (qi + 1) * P],
                            rhs=st["xts"][:, ct, :],
                            start=(ct == 0),
                            stop=(ct == n_ct - 1),
                        )
                        if ct == n_ct - 1:
                            res_write(st[key])
                    return f

                # cold-start HAM pads: dummy matmuls between the DMA-gated
                # steps of QT pass 0 keep the PE active through any stall
                # window so the HAM activity monitor never drops back to
                # half clock (they execute during DMA waits, so ~free; holes
                # must stay <~2us or HAM cools)
                def mk_pad(n, j):
                    def f():
                        ps_d = psN.tile([P, CH], FP32, tag="psN",
                                        name=f"wpad{j}")
                        for i in range(n):
                            mm(ps_d, lhsT=warm[:, :P], rhs=warm,
                               start=(i == 0), stop=(i == n - 1))
                    return f

                # QT: fp8e4 DoubleRow, 256-deep contraction per matmul (8
                # accumulation steps instead of 16; 2x PE throughput)
                nkq = n_ct // 2

                def mk_projq(key, qi, k):
                    def f():
                        if k == 0:
                            st[key] = psX.tile(
                                [P, CH], FP32, tag="psX", name=key
                            )
                        mm(
                            st[key],
                            lhsT=weffqf8_sb[:, 2 * k : 2 * k + 2,
                                            qi * P : (qi + 1) * P],
                            rhs=st["xf8"][:, 2 * k : 2 * k + 2, :],
                            start=(k == 0),
                            stop=(k == nkq - 1),
                            perf_mode=mybir.MatmulPerfMode.DoubleRow,
                        )
                        if k == nkq - 1:
                            cast(qi % 2 == 1, qt_res[qi][:, tok], st[key])
                    return f

                for qi in range(n_ht):
                    for k in range(nkq):
                        out.append(
                            (CH * MM_NS, mk_projq(f"psq{ch}_{qi}", qi, k))
                        )
                        if ch == 0 and qi == 0 and k in (3, 6):
                            out.append((0.0, mk_pad(3, k)))
                for ki in range(n_klt):
                    def wr(ps, ki=ki):
                        c_t = cktp.tile(
                            [P, CH], BF16, tag=f"ckt{ki}", name=f"ckt{ki}_{ch}"
                        )
                        cast(ki % 2 == 1, c_t, ps)
                        st.setdefault("ckt", {})[ki] = c_t
                        # fp8 copy of c_kv for the DoubleRow KT matmul (V
                        # keeps the bf16 copy; v noise would hit the output
                        # directly, k noise only via softmax)
                        if ki == 0:
                            st["ckt8"] = ckt8p.tile(
                                [P, n_klt, CH], FP8, tag="ckt8",
                                name=f"ckt8_{ch}"
                            )
                        # always DVE: keeps the fp8 copy off the scalar
                        # engine, whose exp throughput paces attention
                        cast(False, st["ckt8"][:, ki, :], ps)
                    for ct in range(n_ct):
                        out.append(
                            (CH * MM_NS, mk_proj(f"psc{ch}_{ki}", ki, ct,
                                                 wdkvt_sb, wr))
                        )
                        if ch == 0 and ki == 0 and ct in (3, 7, 11):
                            # CKT pass 0 is gated by the bf16 x0/wdkvt
                            # stream at cold start; pads keep HAM hot
                            # (holes would otherwise reach ~2us and cool it)
                            out.append((0.0, mk_pad(8, 16 + ct)))

                # KT (contraction over kv-latent), fp8 DoubleRow, one head
                # per pass (2 steps of 256-deep contraction)
                def mk_kt(hi, kl2):
                    def f():
                        if kl2 == 0:
                            st[f"psk{hi}"] = psX.tile(
                                [P, CH], FP32, tag="psX", name=f"psk{ch}_{hi}"
                            )
                        mm(
                            st[f"psk{hi}"],
                            lhsT=wukt_sb[:, 2 * kl2 : 2 * kl2 + 2,
                                         hi * P : (hi + 1) * P],
                            rhs=st["ckt8"][:, 2 * kl2 : 2 * kl2 + 2, :],
                            start=(kl2 == 0),
                            stop=(kl2 == n_klt // 2 - 1),
                            perf_mode=mybir.MatmulPerfMode.DoubleRow,
                        )
                        if kl2 == n_klt // 2 - 1:
                            cast(hi % 2 == 1, kt_sb[hi][:, tok], st[f"psk{hi}"])
                    return f

                for hi in range(n_ht):
                    for kl2 in range(n_klt // 2):
                        out.append((CH * MM_NS, mk_kt(hi, kl2)))

                # V chunk: token-major [tok, HD], fp16 for the ctx matmul
                def mk_v(tt):
                    def f():
                        tglob = ch * kpc + tt
                        psv = psX.tile([P, CH], FP32, tag="psX", name=f"psv{tglob}")
                        for kl in range(n_klt):
                            mm(
                                psv[:, :HD],
                                lhsT=st["ckt"][kl][:, tt * P : (tt + 1) * P],
                                rhs=wuvt_sb[:, kl, :],
                                start=(kl == 0),
                                stop=(kl == n_klt - 1),
                            )
                        cast(tt % 2 == 1, v_sb[tglob], psv[:, :HD])
                    return f

                for tt in range(kpc):
                    out.append((n_klt * HD * MM_NS, mk_v(tt)))
                return out

            # ---- output chunk as filler closures ------------------------
            # casts go into a 4-wide staging tile; one batched DMA per 4 cts
            def o_closures(g, otn):
                tok = slice(g * CH, (g + 1) * CH)
                st = {}
                out = []

                def mk(ct):
                    def f():
                        ps_o = psX.tile([P, CH], FP32, tag="psX",
                                        name=f"pso{g}_{ct}")
                        for d in range(n_ht):
                            mm(
                                ps_o,
                                lhsT=wot_sb[:, d, ct * P : (ct + 1) * P],
                                rhs=otn[d],
                                start=(d == 0),
                                stop=(d == n_ht - 1),
                            )
                        if ct % 4 == 0:
                            st["oo"] = oop.tile(
                                [P, 4, CH], BF16, tag="oo", name=f"oo{g}_{ct}"
                            )
                        cast(ct % 2 == 1, st["oo"][:, ct % 4, :], ps_o)
                        if g == n_ch - 1:
                            # last chunk: per-ct DMAs issued right after each
                            # cast, so the tail-critical final transfer is
                            # only 128KB instead of 0.5MB (sync is idle here)
                            nc.sync.dma_start(
                                out=r128(outt[ct * P : (ct + 1) * P, tok]),
                                in_=st["oo"][:, ct % 4 : ct % 4 + 1, :],
                            )
                        elif ct % 4 == 3:
                            nc.sync.dma_start(
                                out=r128(
                                    outt[(ct - 3) * P : (ct + 1) * P, tok]
                                ),
                                in_=st["oo"],
                            )
                    return f

                for ct in range(n_ct):
                    out.append((n_ht * CH * MM_NS, mk(ct)))
                return out

            # ---- filler machinery ---------------------------------------
            fillx = deque()  # barrier class: must drain before next A group
            fillo = deque()  # lazy class: output chunks, no deadline
            pace = [0.0, 0.0]  # budget, spent
            # during the LAST attention group, keep a few o() closures in
            # reserve: they bridge the ~2us norm-chain latency between the
            # last ctx matmul and the first usable o(n_ch-1) matmul, so the
            # PE never idles there (an idle hole would also cool HAM)
            o_reserve = [0]

            def fill(budget_ns):
                # fillx strictly first: projection passes hold psX tiles
                # ACROSS closures, so nothing else may allocate from psX
                # until the pass completes (Tile pools assume emission-order
                # rotation). o closures are each atomic, and otp bufs=4
                # gives them a full kernel of slack to drain late.
                pace[0] += budget_ns
                while pace[1] < pace[0] and (
                    fillx or len(fillo) > o_reserve[0]
                ):
                    ns, fn = (fillx if fillx else fillo).popleft()
                    fn()
                    pace[1] += ns

            def force_x():
                while fillx:
                    ns, fn = fillx.popleft()
                    fn()
                    pace[1] += ns

            # ---- attention ----------------------------------------------
            st = {}
            otn_by_g = {}
            pending = []

            def emit_s(g, h, t):
                hs = st.setdefault((g, h), {"pts": {}})
                j = t - kpc * g
                qoff = max(0, j) * P
                w = CH - qoff  # live q-columns of this tile
                qs = slice(g * CH + qoff, (g + 1) * CH)
                ps_s = psS.tile([P, CH], FP32, tag="psS", name=f"pss{h}_{g}_{t}")
                mm(
                    ps_s[:, :w],
                    lhsT=kt_sb[h][:, t * P : (t + 1) * P],
                    rhs=qt_res[h][:, qs],
                    start=True,
                    stop=True,
                )
                pt = ptp.tile([P, CH], FP16, tag="pt", name=f"pt{h}_{g}_{t}")
                # qt_res carries q*QSCALE and kt_sb carries k*QSCALE (fp8
                # weight scaling); fold the 1/QSCALE^2 into the exp scale
                nc.scalar.activation(
                    out=pt[:, :w],
                    in_=ps_s[:, :w],
                    func=mybir.ActivationFunctionType.Exp,
                    scale=scale / (QSCALE * QSCALE),
                )
                if j >= 0:
                    # keep P^T[kj, q] only where live q-col >= kj row
                    nc.gpsimd.affine_select(
                        out=pt[:, :w],
                        in_=pt[:, :w],
                        compare_op=mybir.AluOpType.is_ge,
                        fill=0.0,
                        base=0,
                        channel_multiplier=-1,
                        pattern=[[1, w]],
                    )
                hs["pts"][t] = (pt, qoff, w)

            def emit_norm(g, h):
                hs = st[(g, h)]
                ps_n = psN.tile([P, CH], FP32, tag="psN", name=f"psn{h}_{g}")
                mm(ps_n, lhsT=ones_sq, rhs=hs["acc"], start=True, stop=True)
                # 1/d as exp(-ln d) on the scalar engine: the iterative DVE
                # InstReciprocal costs 3.3us; Ln/Exp share one activation
                # table so these are two ~0.7us table ops instead
                lntmp = bcp.tile([P, CH], FP32, tag="lntmp", name=f"ln{h}_{g}")
                nc.scalar.activation(
                    out=lntmp, in_=ps_n, func=mybir.ActivationFunctionType.Ln
                )
                bc = bcp.tile([P, CH], FP32, tag="bc", name=f"bc{h}_{g}")
                nc.scalar.activation(
                    out=bc, in_=lntmp,
                    func=mybir.ActivationFunctionType.Exp, scale=-1.0,
                )
                ot_t = otp.tile([P, CH], BF16, tag=f"otn{h}", name=f"otn{h}_{g}")
                nc.vector.tensor_mul(out=ot_t, in0=hs["ot"], in1=bc)
                otn_by_g.setdefault(g, {})[h] = ot_t

            def tick():
                for e in pending[:]:
                    e[0] -= 1
                    if e[0] <= 0:
                        pending.remove(e)
                        e[1]()

            # upfront: projections for chunks 0 and 1 (chunk 0 as a block;
            # chunk 1's KT/V drain as filler inside attention group 0)
            for ns, fn in x_closures(0, xts01[0]):
                fn()
            # open + QT (n_ht passes) + CKT (n_klt passes) upfront; KT/V
            # drain as filler inside attention group 0
            n_up = 1 + n_ht * (n_ct // 2) + n_klt * n_ct
            c1 = x_closures(1, xts01[1])
            for ns, fn in c1[:n_up]:
                fn()
            fillx.extend(c1[n_up:])

            flat = [
                (g, h, t)
                for g in range(n_ch)
                for h in range(n_ht)
                for t in range(kpc * (g + 1))
            ]
            LA = 3
            for si in range(min(LA, len(flat))):
                emit_s(*flat[si])
            cur_g = 0
            for ci, (g, h, t) in enumerate(flat):
                if g != cur_g:
                    cur_g = g
                    force_x()  # X(g) projections must precede A(g)
                    if g + 1 < n_ch:
                        fillx.extend(x_closures(g + 1))
                    else:
                        o_reserve[0] = 3
                nk = kpc * (g + 1)
                hs = st[(g, h)]
                if t == 0:
                    hs["ot"] = psA.tile([P, CH], FP32, tag="psA", name=f"psot{h}_{g}")
                si = ci + LA
                if si < len(flat):
                    emit_s(*flat[si])
                pt, qoff, w = hs["pts"].pop(t)
                # denominator accumulation on the DVE (fp16 4x mode)
                if t == 0:
                    # t==0 always has qoff=0, w=CH: acc fully initialized
                    acc = accp.tile([P, CH], FP16, tag="acc", name=f"acc{h}_{g}")
                    hs["acc"] = acc
                    nc.vector.tensor_copy(out=acc, in_=pt)
                else:
                    nc.vector.tensor_add(
                        out=hs["acc"][:, qoff:], in0=hs["acc"][:, qoff:], in1=pt[:, :w]
                    )
                mm(
                    hs["ot"][:, qoff : qoff + w],
                    lhsT=v_sb[t][:, h * P : (h + 1) * P],
                    rhs=pt[:, :w],
                    start=(t == 0),
                    stop=(t == nk - 1),
                )
                if t == nk - 1:
                    pending.append([2, (lambda gg=g, hh=h: emit_norm(gg, hh))])
                    if h == n_ht - 1:
                        pending.append(
                            [4, (lambda gg=g: fillo.extend(
                                o_closures(gg, otn_by_g[gg])))]
                        )
                tick()
                fill(180 + 1.05 * w)  # scalar exp pace for this step
            while pending:
                e = pending.pop(0)
                e[1]()
            force_x()
            while fillo:
                ns, fn = fillo.popleft()
                fn()

    if split:
        # for walrus only; CoreSim's race detector can't see the added NOPs
        split_multi_waits(nc)
    return nc


# ---------------------------------------------------------------------------
# Host side
# ---------------------------------------------------------------------------
B, N, D_IN = 2, 2048, 2048
D_OUT, N_HEADS = 2048, 16
D_C_KV, D_C_Q = 512, 2048
N_CORES = 8
HG = 4  # head-groups
HD = D_OUT // HG  # 512 dims per head-group

_NC_CACHE = {}


def _get_nc():
    if "nc" not in _NC_CACHE:
        _NC_CACHE["nc"] = build_nc(
            N=N, D=D_IN, KV=D_C_KV, HC=N_HEADS // HG, DH=D_OUT // N_HEADS
        )
    return _NC_CACHE["nc"]


def make_in_maps(x, W_DQ, W_UQ, W_DKV, W_UK, W_UV, W_O):
    import ml_dtypes

    bf = ml_dtypes.bfloat16
    f8 = ml_dtypes.float8_e4m3fn
    c = np.ascontiguousarray

    def cb(a):
        return c(np.asarray(a, np.float32)).astype(bf)

    xtb = [cb(np.asarray(x[b], np.float32).T) for b in range(B)]
    xf8b = [
        c(np.asarray(x[b], np.float32).T).astype(f8) for b in range(B)
    ]
    wdq32 = np.asarray(W_DQ, np.float32)
    wuq32 = np.asarray(W_UQ, np.float32)
    wdkvt = cb(np.asarray(W_DKV, np.float32).T)
    in_maps = []
    weffq_by_hg = {}
    for core in range(N_CORES):
        b, hg = divmod(core, HG)
        hs = slice(hg * HD, (hg + 1) * HD)
        if hg not in weffq_by_hg:
            # weight absorption (host, fp32): W_effQ^T = W_DQ^T @ W_UQ_hg^T
            # scaled by QSCALE into fp8e4's normal range (device folds the
            # 1/QSCALE into the softmax exp scale)
            weffq_by_hg[hg] = c(
                (wdq32.T @ wuq32[hs, :].T) * QSCALE
            ).astype(f8)
        in_maps.append(
            {
                "xt": xtb[b],
                "xf8": xf8b[b],
                "weffqf8": weffq_by_hg[hg],
                "wdkvt": wdkvt,
                "wukt": c(
                    np.asarray(W_UK, np.float32)[hs, :].T * QSCALE
                ).astype(f8),
                "wuvt": cb(np.asarray(W_UV, np.float32)[hs, :].T),
                "wot": cb(np.asarray(W_O, np.float32)[:, hs].T),
            }
        )
    return in_maps


def kernel(x, W_DQ, W_UQ, W_DKV, W_UK, W_UV, W_O, b_O, _run_kwargs=None):
    nc = _get_nc()
    in_maps = make_in_maps(x, W_DQ, W_UQ, W_DKV, W_UK, W_UV, W_O)
    res = run_bass_kernel_spmd(
        nc, in_maps, list(range(N_CORES)), **(_run_kwargs or {})
    )
    out = np.zeros((B, N, D_IN), np.float32)
    for core in range(N_CORES):
        b = core // HG
        out[b] += res.results[core]["outt"].T.astype(np.float32)
    out += np.asarray(b_O, np.float32)[None, None, :]
    if _run_kwargs is not None:
        _NC_CACHE["last_results"] = res
    return out



# revision 39
# speedup vs baseline: 1.0150x; 1.0150x over previous
"""Multi-Head Latent Attention (MLA) on 8 Trainium2 NeuronCores.

Sharding: core = b*4 + hg, b in {0,1} batch, hg in 0..3 head-groups of 4
heads (512 of the 2048 d_out dims). The latent projections (c_kv) are
computed per-core; the low-rank Q path is absorbed ON HOST:
    W_effQ^T = W_DQ^T @ W_UQ_shard^T   ([d_in, 512])
(a weights-only transform), so the device does q_shard = x_b @ W_effQ as
one 2048-contraction matmul and never sees W_DQ/W_UQ.

Everything on device lives in transposed "feature-on-partition" layout:
  XT = x[b]^T [d_in, N], QT = q^T, CKT = c_kv^T, KT = k^T. Attention
computes S^T tiles [ktok, qtok] directly (matmul lhsT=KT-slice,
rhs=QT-slice), so softmax probabilities come out of exp already in the
layout the ctx matmul needs (contraction over ktok on partitions) — no
PE transposes. Causality: affine_select zeroes P^T[kj, q] for kj > q
after exp (no max-subtraction needed: scores are O(1) by construction).

The softmax denominator is NOT a per-tile PE matmul: exp tiles (fp16)
are accumulated on the DVE (fp16 all-2-byte => 4x mode), then ONE
all-ones [128,128] matmul per (group, head) broadcasts the partition
sums to every partition; reciprocal+multiply normalize ctx^T straight
into the per-(g,h) normalized-ctx tile the output matmul reads.

Scheduling: the attention inner loop is paced by the scalar-engine exp
(~0.7us per [128,512] tile) while its own PE work (S+ctx) is only
~0.43us. A filler queue of projection-chunk and output-chunk closures
is drained between attention steps on a ns budget, so the PE stays fed
during the scalar-bound attention stretches instead of idling.

Output per core: partial out^T [d_in, N] (contraction over this core's
512 ctx dims); host sums the 4 head-group partials per batch and adds
the bias.
"""

import math
from collections import deque
from contextlib import ExitStack

import numpy as np

import concourse.bass as bass
import concourse.bass_isa as bass_isa
import concourse.mybir as mybir
import concourse.tile as tile
from concourse.bass_utils import run_bass_kernel_spmd
from concourse.vector_clock import ScopedClock, VectorClock

FP32 = mybir.dt.float32
BF16 = mybir.dt.bfloat16
FP16 = mybir.dt.float16
FP8 = mybir.dt.float8e4
P = 128
CH = 512
CAST_SPLIT = True
# The absorbed Q weights are scaled by QSCALE before the fp8e4 cast so they
# sit in e4m3's normal range (raw std ~0.018 straddles the 2^-6 subnormal
# cutoff); the 1/QSCALE is folded into the softmax exp scale.
QSCALE = 64.0


class SplitDrainTileContext(tile.TileContext):
    """TileContext whose tail drain splits sem waits across multiple NOPs.

    The walrus build in this container rejects instructions carrying >2
    sync waits ("Too many sync wait commands"); stock TileContext puts a
    wait for every outstanding proc on the single kernel-tail drain.
    """

    def _drain_and_barrier(self, tick_clock, wait_clock):
        g = tick_clock.global_clock
        n = len(g)
        for i in range(n):
            t = g[i]
            if t <= 0:
                continue
            vc = VectorClock([0] * n)
            vc.require_at_least(i, t)
            nop = self.nc.sync.nop(hint="split_drain_wait", nofuse=True)
            wait_clock.add_sem_waits(nop.ins, ScopedClock({None: vc}))
        self.nc.sync.drain()
        self.nc.all_engine_barrier()
        assert self.sems is not None
        popped = self.nc._tile_sem_poison_stack.pop()
        assert popped is self._sem_poison
        self.nc.clear_and_free_semaphores(list(self.sems.allocated().values()))
        self.nc.all_engine_barrier()


def split_multi_waits(nc, max_waits=1):
    """Hoist extra sync waits onto same-engine NOPs.

    The walrus build here rejects instructions with more than ~2 sync wait
    commands; Tile freely attaches one wait per outstanding proc. An engine
    executes its stream in order, so a NOP carrying a wait immediately
    before the instruction is semantically identical.
    """
    for fn in nc.m.functions:
        for bb in fn.blocks:
            new_insts = []
            changed = False
            for inst in bb.instructions:
                si = inst.sync_info
                waits = list(si.on_wait) if si is not None else []
                if len(waits) > max_waits:
                    extra, keep = waits[:-max_waits], waits[-max_waits:]
                    for k, w in enumerate(extra):
                        nop = mybir.InstNoOp(
                            name=f"{inst.name}.w{k}",
                            sync_info=mybir.SyncInfo(on_wait=[w], on_update=[]),
                            bass_nofuse=True,
                            engine=inst.engine,
                        )
                        new_insts.append(nop)
                    inst.sync_info = mybir.SyncInfo(
                        on_wait=keep, on_update=list(si.on_update)
                    )
                    changed = True
                new_insts.append(inst)
            if changed:
                bb.instructions = new_insts


def build_nc(N=2048, D=2048, KV=512, HC=4, DH=128, split=True):
    """Build the per-core Bass program (identical on all 8 cores)."""
    HD = HC * DH  # this core's slice of d_out
    n_ct = D // P  # d_in partition tiles
    n_klt = KV // P  # kv-latent tiles
    n_ht = HD // P  # head tiles (DH == P so one tile per head)
    n_ch = N // CH  # token chunks
    kpc = CH // P  # ktiles per chunk (4)
    scale = 1.0 / math.sqrt(DH)
    assert DH == P and n_ct % 4 == 0

    nc = bass.Bass("TRN2", target_bir_lowering=False, debug=False)
    xt = nc.declare_dram_parameter("xt", [D, N], BF16, isOutput=False)
    # Q path in fp8e4 DoubleRow (2x PE throughput, 256-deep contraction per
    # matmul). q errors only reach the output through softmax probabilities,
    # so fp8's ~3.7% q-noise decays to ~1% on the final output — well under
    # the 2e-2 gate. x/weffq ship as separate fp8 copies from the host.
    xf8 = nc.declare_dram_parameter("xf8", [D, N], FP8, isOutput=False)
    weffqf8 = nc.declare_dram_parameter("weffqf8", [D, HD], FP8, isOutput=False)
    wdkvt = nc.declare_dram_parameter("wdkvt", [D, KV], BF16, isOutput=False)
    # K path also fp8 (k errors, like q errors, only reach the output
    # through softmax probabilities); W_UK ships x QSCALE like weffq
    wukt = nc.declare_dram_parameter("wukt", [KV, HD], FP8, isOutput=False)
    wuvt = nc.declare_dram_parameter("wuvt", [KV, HD], BF16, isOutput=False)
    wot = nc.declare_dram_parameter("wot", [HD, D], BF16, isOutput=False)
    outt = nc.declare_dram_parameter("outt", [D, N], BF16, isOutput=True)

    with SplitDrainTileContext(nc) as tc, ExitStack() as top:
        mm = nc.tensor.matmul

        # ~4us of dummy matmuls at kernel start: trips the HAM activity
        # window so the real matmuls start at 2.4GHz instead of 1.2.
        const = top.enter_context(tc.tile_pool(name="const", bufs=1))
        # all-ones square: one matmul broadcasts the partition-sum of the
        # softmax-denominator accumulator to every partition
        # (gpsimd.partition_all_reduce would free the PE here, but this
        # walrus build rejects InstPartitionAllReduce: "ISA wrong length")
        ones_sq = const.tile([P, P], FP16, tag="ones_sq", name="ones_sq")
        nc.vector.memset(ones_sq, 1.0)
        # Dummy matmuls trip the HAM activity window so real matmuls run at
        # full speed. The startup is DMA-bound (x0+weights stream at
        # ~350GB/s until ~25us), and any PE-idle hole >~2us mid-startup
        # makes HAM drop back to half clock — so beyond this upfront block,
        # more dummies are PADDED between the DMA-gated g4-blocks of the
        # first QT pass (see x_closures pad hook below).
        N_WARM = 10
        warm = const.tile([P, CH], BF16, tag="warm", name="warm")
        nc.vector.memset(warm, 0.0)
        with tc.tile_pool(name="psWarm", bufs=1, space="PSUM") as psWarm:
            wps = psWarm.tile([P, CH], FP32, tag="wps", name="wps")
            for i in range(N_WARM):
                mm(wps, lhsT=warm[:, :P], rhs=warm,
                   start=(i == 0), stop=(i == N_WARM - 1))

        # whole-kernel residents (bf16/fp16 keeps this under SBUF budget)
        kvp = top.enter_context(tc.tile_pool(name="kv", bufs=1))
        kt_sb = [
            kvp.tile([P, N], BF16, tag=f"kt{h}", name=f"kt{h}") for h in range(n_ht)
        ]
        v_sb = [
            kvp.tile([P, HD], FP16, tag=f"v{t}", name=f"v{t}")
            for t in range(N // P)
        ]
        qt_res = [
            kvp.tile([P, N], BF16, tag=f"qt{h}", name=f"qt{h}") for h in range(n_ht)
        ]

        # weights as single wide tiles: batched DMAs (each dma_start costs
        # ~600ns of Sync-engine issue time, so fewer+bigger is better)
        wp = top.enter_context(tc.tile_pool(name="wp", bufs=1))
        weffqf8_sb = wp.tile([P, n_ct, HD], FP8, tag="weffqf8", name="weffqf8")
        wdkvt_sb = wp.tile([P, n_ct, KV], BF16, tag="wdkvt", name="wdkvt")
        wukt_sb = wp.tile([P, n_klt, HD], FP8, tag="wukt", name="wukt")
        wuvt_sb = wp.tile([P, n_klt, HD], BF16, tag="wuvt", name="wuvt")
        wot_sb = wp.tile([P, n_ht, D], BF16, tag="wot", name="wot")

        def r128(ap):
            return ap.rearrange("(a p) c -> p a c", p=P)

        with (
            tc.tile_pool(name="xtp", bufs=2) as xtp,
            tc.tile_pool(name="xf8p", bufs=2) as xf8p,
            tc.tile_pool(name="cktp", bufs=2) as cktp,
            tc.tile_pool(name="ckt8p", bufs=2) as ckt8p,
            tc.tile_pool(name="ptp", bufs=6) as ptp,
            tc.tile_pool(name="accp", bufs=2) as accp,
            tc.tile_pool(name="bcp", bufs=2) as bcp,
            tc.tile_pool(name="otp", bufs=4) as otp,
            tc.tile_pool(name="oop", bufs=2) as oop,
            tc.tile_pool(name="psX", bufs=2, space="PSUM") as psX,
            tc.tile_pool(name="psS", bufs=3, space="PSUM") as psS,
            tc.tile_pool(name="psA", bufs=2, space="PSUM") as psA,
            tc.tile_pool(name="psN", bufs=1, space="PSUM") as psN,
        ):
            def dma_xt(ch):
                tok = slice(ch * CH, (ch + 1) * CH)
                xts = xtp.tile([P, n_ct, CH], BF16, tag="xts", name=f"xts{ch}")
                xf8s = xf8p.tile([P, n_ct, CH], FP8, tag="xf8s",
                                 name=f"xf8s{ch}")
                nc.sync.dma_start(out=xf8s, in_=r128(xf8[:, tok]))
                for g4 in range(0, n_ct, 4):
                    nc.sync.dma_start(
                        out=xts[:, g4 : g4 + 4, :],
                        in_=r128(xt[g4 * P : (g4 + 4) * P, tok]),
                    )
                return xts, xf8s

            # startup DMAs, dual HWDGE streams (sync | scalar) balanced so
            # the aggregate ~350GB/s serves data in CONSUMPTION order, with
            # per-g4-block DMAs so Tile gates each matmul on its own block:
            #   sync:   xf8(c0) blocks  -> xt(c0) b0..b2 -> wukt -> xt b3
            #           -> chunk1 -> wot
            #   scalar: weffqf8 blocks  -> wdkvt b0..b2 -> wuvt -> wdkvt b3
            xts01 = {}
            xts0 = xtp.tile([P, n_ct, CH], BF16, tag="xts", name="xts0")
            xf80 = xf8p.tile([P, n_ct, CH], FP8, tag="xf8s", name="xf8s0")
            for g4 in range(0, n_ct, 4):
                nc.sync.dma_start(
                    out=xf80[:, g4 : g4 + 4, :],
                    in_=r128(xf8[g4 * P : (g4 + 4) * P, 0:CH]),
                )
                nc.scalar.dma_start(
                    out=weffqf8_sb[:, g4 : g4 + 4, :],
                    in_=r128(weffqf8[g4 * P : (g4 + 4) * P, :]),
                )
            for i, g4 in enumerate(range(0, n_ct, 4)):
                if i == 3:
                    # small KT/V weights slot in before the last CKT pair so
                    # the KT/V passes stream right behind CKT
                    nc.sync.dma_start(out=wukt_sb, in_=r128(wukt[:, :]))
                    nc.scalar.dma_start(out=wuvt_sb, in_=r128(wuvt[:, :]))
                nc.sync.dma_start(
                    out=xts0[:, g4 : g4 + 4, :],
                    in_=r128(xt[g4 * P : (g4 + 4) * P, 0:CH]),
                )
                nc.scalar.dma_start(
                    out=wdkvt_sb[:, g4 : g4 + 4, :],
                    in_=r128(wdkvt[g4 * P : (g4 + 4) * P, :]),
                )
            xts01[0] = (xts0, xf80)
            xts01[1] = dma_xt(1)
            nc.sync.dma_start(out=wot_sb, in_=r128(wot[:, :]))

            MM_NS = 0.43  # ns per moving column, one 128-contraction matmul

            def cast(alt, out, in_):
                # split PSUM->SBUF casts between the two copy-capable
                # engines; Copy shares the exp activation table, so no
                # table reloads are triggered on the scalar engine
                if alt and CAST_SPLIT:
                    nc.scalar.copy(out=out, in_=in_)
                else:
                    nc.vector.tensor_copy(out=out, in_=in_)

            # ---- projection chunk as filler closures --------------------
            def x_closures(ch, pre=None):
                """Closures computing QT/CKT/KT/V for token chunk ch."""
                tok = slice(ch * CH, (ch + 1) * CH)
                st = {}
                out = []

                def open_chunk():
                    st["xts"], st["xf8"] = pre if pre is not None else dma_xt(ch)

                out.append((0, open_chunk))

                # QT then CKT: SINGLE-accumulator passes over the 16 d_in
                # tiles -- a new pass's psX alloc then sits a full pass
                # (~3.5us) behind the previous pass's final cast, so the
                # pool rotation never stalls the PE at pass boundaries
                def mk_proj(key, qi, ct, wt, res_write):
                    def f():
                        if ct == 0:
                            st[key] = psX.tile(
                                [P, CH], FP32, tag="psX", name=key
                            )
                        mm(
                            st[key],
                            lhsT=wt[:, ct, qi * P : (qi + 1) * P],
                            rhs=st["xts"][:, ct, :],
                            start=(ct == 0),
                            stop=(ct == n_ct - 1),
                        )
                        if ct == n_ct - 1:
                            res_write(st[key])
                    return f

                # cold-start HAM pads: dummy matmuls between the DMA-gated
                # steps of QT pass 0 keep the PE active through any stall
                # window so the HAM activity monitor never drops back to
                # half clock (they execute during DMA waits, so ~free; holes
                # must stay <~2us or HAM cools)
                def mk_pad(n, j):
                    def f():
                        ps_d = psN.tile([P, CH], FP32, tag="psN",
                                        name=f"wpad{j}")
                        for i in range(n):
                            mm(ps_d, lhsT=warm[:, :P], rhs=warm,
                               start=(i == 0), stop=(i == n - 1))
                    return f

                # QT: fp8e4 DoubleRow, 256-deep contraction per matmul (8
                # accumulation steps instead of 16; 2x PE throughput)
                nkq = n_ct // 2

                def mk_projq(key, qi, k):
                    def f():
                        if k == 0:
                            st[key] = psX.tile(
                                [P, CH], FP32, tag="psX", name=key
                            )
                        mm(
                            st[key],
                            lhsT=weffqf8_sb[:, 2 * k : 2 * k + 2,
                                            qi * P : (qi + 1) * P],
                            rhs=st["xf8"][:, 2 * k : 2 * k + 2, :],
                            start=(k == 0),
                            stop=(k == nkq - 1),
                            perf_mode=mybir.MatmulPerfMode.DoubleRow,
                        )
                        if k == nkq - 1:
                            cast(qi % 2 == 1, qt_res[qi][:, tok], st[key])
                    return f

                for qi in range(n_ht):
                    for k in range(nkq):
                        out.append(
                            (CH * MM_NS, mk_projq(f"psq{ch}_{qi}", qi, k))
                        )
                        if ch == 0 and qi == 0 and k in (3, 6):
                            out.append((0.0, mk_pad(3, k)))
                for ki in range(n_klt):
                    def wr(ps, ki=ki):
                        c_t = cktp.tile(
                            [P, CH], BF16, tag=f"ckt{ki}", name=f"ckt{ki}_{ch}"
                        )
                        cast(ki % 2 == 1, c_t, ps)
                        st.setdefault("ckt", {})[ki] = c_t
                        # fp8 copy of c_kv for the DoubleRow KT matmul (V
                        # keeps the bf16 copy; v noise would hit the output
                        # directly, k noise only via softmax)
                        if ki == 0:
                            st["ckt8"] = ckt8p.tile(
                                [P, n_klt, CH], FP8, tag="ckt8",
                                name=f"ckt8_{ch}"
                            )
                        # always DVE: keeps the fp8 copy off the scalar
                        # engine, whose exp throughput paces attention
                        cast(False, st["ckt8"][:, ki, :], ps)
                    for ct in range(n_ct):
                        out.append(
                            (CH * MM_NS, mk_proj(f"psc{ch}_{ki}", ki, ct,
                                                 wdkvt_sb, wr))
                        )
                        if ch == 0 and ki == 0 and ct in (3, 7, 11):
                            # CKT pass 0 is gated by the bf16 x0/wdkvt
                            # stream at cold start; pads keep HAM hot
                            # (holes would otherwise reach ~2us and cool it)
                            out.append((0.0, mk_pad(8, 16 + ct)))

                # KT (contraction over kv-latent), fp8 DoubleRow, one head
                # per pass (2 steps of 256-deep contraction)
                def mk_kt(hi, kl2):
                    def f():
                        if kl2 == 0:
                            st[f"psk{hi}"] = psX.tile(
                                [P, CH], FP32, tag="psX", name=f"psk{ch}_{hi}"
                            )
                        mm(
                            st[f"psk{hi}"],
                            lhsT=wukt_sb[:, 2 * kl2 : 2 * kl2 + 2,
                                         hi * P : (hi + 1) * P],
                            rhs=st["ckt8"][:, 2 * kl2 : 2 * kl2 + 2, :],
                            start=(kl2 == 0),
                            stop=(kl2 == n_klt // 2 - 1),
                            perf_mode=mybir.MatmulPerfMode.DoubleRow,
                        )
                        if kl2 == n_klt // 2 - 1:
                            cast(hi % 2 == 1, kt_sb[hi][:, tok], st[f"psk{hi}"])
                    return f

                for hi in range(n_ht):
                    for kl2 in range(n_klt // 2):
                        out.append((CH * MM_NS, mk_kt(hi, kl2)))

                # V chunk: token-major [tok, HD], fp16 for the ctx matmul
                def mk_v(tt):
                    def f():
                        tglob = ch * kpc + tt
                        psv = psX.tile([P, CH], FP32, tag="psX", name=f"psv{tglob}")
                        for kl in range(n_klt):
                            mm(
                                psv[:, :HD],
                                lhsT=st["ckt"][kl][:, tt * P : (tt + 1) * P],
                                rhs=wuvt_sb[:, kl, :],
                                start=(kl == 0),
                                stop=(kl == n_klt - 1),
                            )
                        cast(tt % 2 == 1, v_sb[tglob], psv[:, :HD])
                    return f

                for tt in range(kpc):
                    out.append((n_klt * HD * MM_NS, mk_v(tt)))
                return out

            # ---- output chunk as filler closures ------------------------
            # casts go into a 4-wide staging tile; one batched DMA per 4 cts
            def o_closures(g, otn):
                tok = slice(g * CH, (g + 1) * CH)
                st = {}
                out = []

                def mk(ct):
                    def f():
                        ps_o = psX.tile([P, CH], FP32, tag="psX",
                                        name=f"pso{g}_{ct}")
                        for d in range(n_ht):
                            mm(
                                ps_o,
                                lhsT=wot_sb[:, d, ct * P : (ct + 1) * P],
                                rhs=otn[d],
                                start=(d == 0),
                                stop=(d == n_ht - 1),
                            )
                        if ct % 4 == 0:
                            st["oo"] = oop.tile(
                                [P, 4, CH], BF16, tag="oo", name=f"oo{g}_{ct}"
                            )
                        cast(ct % 2 == 1, st["oo"][:, ct % 4, :], ps_o)
                        if g == n_ch - 1:
                            # last chunk: per-ct DMAs issued right after each
                            # cast, so the tail-critical final transfer is
                            # only 128KB instead of 0.5MB (sync is idle here)
                            nc.sync.dma_start(
                                out=r128(outt[ct * P : (ct + 1) * P, tok]),
                                in_=st["oo"][:, ct % 4 : ct % 4 + 1, :],
                            )
                        elif ct % 4 == 3:
                            nc.sync.dma_start(
                                out=r128(
                                    outt[(ct - 3) * P : (ct + 1) * P, tok]
                                ),
                                in_=st["oo"],
                            )
                    return f

                for ct in range(n_ct):
                    out.append((n_ht * CH * MM_NS, mk(ct)))
                return out

            # ---- filler machinery ---------------------------------------
            fillx = deque()  # barrier class: must drain before next A group
            fillo = deque()  # lazy class: output chunks, no deadline
            pace = [0.0, 0.0]  # budget, spent
            # during the LAST attention group, keep a few o() closures in
            # reserve: they bridge the ~2us norm-chain latency between the
            # last ctx matmul and the first usable o(n_ch-1) matmul, so the
            # PE never idles there (an idle hole would also cool HAM)
            o_reserve = [0]

            def fill(budget_ns):
                # fillx strictly first: projection passes hold psX tiles
                # ACROSS closures, so nothing else may allocate from psX
                # until the pass completes (Tile pools assume emission-order
                # rotation). o closures are each atomic, and otp bufs=4
                # gives them a full kernel of slack to drain late.
                pace[0] += budget_ns
                while pace[1] < pace[0] and (
                    fillx or len(fillo) > o_reserve[0]
                ):
                    ns, fn = (fillx if fillx else fillo).popleft()
                    fn()
                    pace[1] += ns

            def force_x():
                while fillx:
                    ns, fn = fillx.popleft()
                    fn()
                    pace[1] += ns

            # ---- attention ----------------------------------------------
            st = {}
            otn_by_g = {}
            pending = []

            def emit_s(g, h, t):
                hs = st.setdefault((g, h), {"pts": {}})
                j = t - kpc * g
                qoff = max(0, j) * P
                w = CH - qoff  # live q-columns of this tile
                qs = slice(g * CH + qoff, (g + 1) * CH)
                ps_s = psS.tile([P, CH], FP32, tag="psS", name=f"pss{h}_{g}_{t}")
                mm(
                    ps_s[:, :w],
                    lhsT=kt_sb[h][:, t * P : (t + 1) * P],
                    rhs=qt_res[h][:, qs],
                    start=True,
                    stop=True,
                )
                pt = ptp.tile([P, CH], FP16, tag="pt", name=f"pt{h}_{g}_{t}")
                # qt_res carries q*QSCALE and kt_sb carries k*QSCALE (fp8
                # weight scaling); fold the 1/QSCALE^2 into the exp scale
                nc.scalar.activation(
                    out=pt[:, :w],
                    in_=ps_s[:, :w],
                    func=mybir.ActivationFunctionType.Exp,
                    scale=scale / (QSCALE * QSCALE),
                )
                if j >= 0:
                    # keep P^T[kj, q] only where live q-col >= kj row
                    nc.gpsimd.affine_select(
                        out=pt[:, :w],
                        in_=pt[:, :w],
                        compare_op=mybir.AluOpType.is_ge,
                        fill=0.0,
                        base=0,
                        channel_multiplier=-1,
                        pattern=[[1, w]],
                    )
                hs["pts"][t] = (pt, qoff, w)

            def emit_norm(g, h):
                hs = st[(g, h)]
                ps_n = psN.tile([P, CH], FP32, tag="psN", name=f"psn{h}_{g}")
                mm(ps_n, lhsT=ones_sq, rhs=hs["acc"], start=True, stop=True)
                # 1/d as exp(-ln d) on the scalar engine: the iterative DVE
                # InstReciprocal costs 3.3us; Ln/Exp share one activation
                # table so these are two ~0.7us table ops instead
                lntmp = bcp.tile([P, CH], FP32, tag="lntmp", name=f"ln{h}_{g}")
                nc.scalar.activation(
                    out=lntmp, in_=ps_n, func=mybir.ActivationFunctionType.Ln
                )
                bc = bcp.tile([P, CH], FP32, tag="bc", name=f"bc{h}_{g}")
                nc.scalar.activation(
                    out=bc, in_=lntmp,
                    func=mybir.ActivationFunctionType.Exp, scale=-1.0,
                )
                ot_t = otp.tile([P, CH], BF16, tag=f"otn{h}", name=f"otn{h}_{g}")
                nc.vector.tensor_mul(out=ot_t, in0=hs["ot"], in1=bc)
                otn_by_g.setdefault(g, {})[h] = ot_t

            def tick():
                for e in pending[:]:
                    e[0] -= 1
                    if e[0] <= 0:
                        pending.remove(e)
                        e[1]()

            # upfront: projections for chunks 0 and 1 (chunk 0 as a block;
            # chunk 1's KT/V drain as filler inside attention group 0)
            for ns, fn in x_closures(0, xts01[0]):
                fn()
            # open + QT (n_ht passes) + CKT (n_klt passes) upfront; KT/V
            # drain as filler inside attention group 0
            n_up = 1 + n_ht * (n_ct // 2) + n_klt * n_ct
            c1 = x_closures(1, xts01[1])
            for ns, fn in c1[:n_up]:
                fn()
            fillx.extend(c1[n_up:])

            flat = [
                (g, h, t)
                for g in range(n_ch)
                for h in range(n_ht)
                for t in range(kpc * (g + 1))
            ]
            LA = 3
            for si in range(min(LA, len(flat))):
                emit_s(*flat[si])
            cur_g = 0
            for ci, (g, h, t) in enumerate(flat):
                if g != cur_g:
                    cur_g = g
                    force_x()  # X(g) projections must precede A(g)
                    if g + 1 < n_ch:
                        fillx.extend(x_closures(g + 1))
                    else:
                        o_reserve[0] = 3
                nk = kpc * (g + 1)
                hs = st[(g, h)]
                if t == 0:
                    hs["ot"] = psA.tile([P, CH], FP32, tag="psA", name=f"psot{h}_{g}")
                si = ci + LA
                if si < len(flat):
                    emit_s(*flat[si])
                pt, qoff, w = hs["pts"].pop(t)
                # denominator accumulation on the DVE (fp16 4x mode)
                if t == 0:
                    # t==0 always has qoff=0, w=CH: acc fully initialized
                    acc = accp.tile([P, CH], FP16, tag="acc", name=f"acc{h}_{g}")
                    hs["acc"] = acc
                    nc.vector.tensor_copy(out=acc, in_=pt)
                else:
                    nc.vector.tensor_add(
                        out=hs["acc"][:, qoff:], in0=hs["acc"][:, qoff:], in1=pt[:, :w]
                    )
                mm(
                    hs["ot"][:, qoff : qoff + w],
                    lhsT=v_sb[t][:, h * P : (h + 1) * P],
                    rhs=pt[:, :w],
                    start=(t == 0),
                    stop=(t == nk - 1),
                )
                if t == nk - 1:
                    pending.append([2, (lambda gg=g, hh=h: emit_norm(gg, hh))])
                    if h == n_ht - 1:
                        pending.append(
                            [4, (lambda gg=g: fillo.extend(
                                o_closures(gg, otn_by_g[gg])))]
                        )
                tick()
                fill(180 + 1.05 * w)  # scalar exp pace for this step
            while pending:
                e = pending.pop(0)
                e[1]()
            force_x()
            while fillo:
                ns, fn = fillo.popleft()
                fn()

    if split:
        # for walrus only; CoreSim's race detector can't see the added NOPs
        split_multi_waits(nc)
    return nc


# ---------------------------------------------------------------------------
# Host side
# ---------------------------------------------------------------------------
B, N, D_IN = 2, 2048, 2048
D_OUT, N_HEADS = 2048, 16
D_C_KV, D_C_Q = 512, 2048
N_CORES = 8
HG = 4  # head-groups
HD = D_OUT // HG  # 512 dims per head-group

_NC_CACHE = {}


def _get_nc():
    if "nc" not in _NC_CACHE:
        _NC_CACHE["nc"] = build_nc(
            N=N, D=D_IN, KV=D_C_KV, HC=N_HEADS // HG, DH=D_OUT // N_HEADS
        )
    return _NC_CACHE["nc"]


def make_in_maps(x, W_DQ, W_UQ, W_DKV, W_UK, W_UV, W_O):
    import ml_dtypes

    bf = ml_dtypes.bfloat16
    f8 = ml_dtypes.float8_e4m3fn
    c = np.ascontiguousarray

    def cb(a):
        return c(np.asarray(a, np.float32)).astype(bf)

    xtb = [cb(np.asarray(x[b], np.float32).T) for b in range(B)]
    xf8b = [
        c(np.asarray(x[b], np.float32).T).astype(f8) for b in range(B)
    ]
    wdq32 = np.asarray(W_DQ, np.float32)
    wuq32 = np.asarray(W_UQ, np.float32)
    wdkvt = cb(np.asarray(W_DKV, np.float32).T)
    in_maps = []
    weffq_by_hg = {}
    for core in range(N_CORES):
        b, hg = divmod(core, HG)
        hs = slice(hg * HD, (hg + 1) * HD)
        if hg not in weffq_by_hg:
            # weight absorption (host, fp32): W_effQ^T = W_DQ^T @ W_UQ_hg^T
            # scaled by QSCALE into fp8e4's normal range (device folds the
            # 1/QSCALE into the softmax exp scale)
            weffq_by_hg[hg] = c(
                (wdq32.T @ wuq32[hs, :].T) * QSCALE
            ).astype(f8)
        in_maps.append(
            {
                "xt": xtb[b],
                "xf8": xf8b[b],
                "weffqf8": weffq_by_hg[hg],
                "wdkvt": wdkvt,
                "wukt": c(
                    np.asarray(W_UK, np.float32)[hs, :].T * QSCALE
                ).astype(f8),
                "wuvt": cb(np.asarray(W_UV, np.float32)[hs, :].T),
                "wot": cb(np.asarray(W_O, np.float32)[:, hs].T),
            }
        )
    return in_maps


def kernel(x, W_DQ, W_UQ, W_DKV, W_UK, W_UV, W_O, b_O, _run_kwargs=None):
    nc = _get_nc()
    in_maps = make_in_maps(x, W_DQ, W_UQ, W_DKV, W_UK, W_UV, W_O)
    res = run_bass_kernel_spmd(
        nc, in_maps, list(range(N_CORES)), **(_run_kwargs or {})
    )
    out = np.zeros((B, N, D_IN), np.float32)
    for core in range(N_CORES):
        b = core // HG
        out[b] += res.results[core]["outt"].T.astype(np.float32)
    out += np.asarray(b_O, np.float32)[None, None, :]
    if _run_kwargs is not None:
        _NC_CACHE["last_results"] = res
    return out

